# revision 29
# baseline (speedup 1.0000x reference)
"""BiUTE kernel for Trainium2, 8-core data-parallel over batch.

Math (per batch element b, T=128, N=12, D=1024, F=2D=2048):
  u = Wq.sum(0)                                  [D]
  w[t,n]  = sum_d feat[t,n,d] * u[d]             [T,N]
  g[t,d]  = sum_n w[t,n] * feat[t,n,d]           [T,D]
  f[t,d]  = max_n feat[t,n,d]                    [T,D]
  n = [g | f]                                    [T,F]
  tb = n @ Wtb.T ; pb = n @ Wpb.T ; gb = n @ Wgb.T
  sb = (tb @ pb.T) * scale ; out_b = (sb*lower) @ gb
  (same for 'after' branch with upper mask)
  out = n + out_b + out_a                        [T,F]

Sharding: B=16 split 2 per core across 8 cores; weights replicated.

Precision strategy: the g-half of n has sigma~16.5 vs the f-half's
~1.7, so the f-half of every projection contraction runs in fp8-e4m3
with DoubleRow (double-pumped) matmuls -- 2x PE rate and half the DMA
bytes for those weight rows -- while the g-half stays fp16.  Validated
rel-err ~1.2e-2 vs the 2e-2 gate (fp16 baseline 1.0e-3).

Prologue: w via DVE STT accum, diag(w_c) built on ACT (Copy with
per-partition scale), g accumulated on PE chunk-paced with the feat DMA
stream (keeps HAM warm), f via DVE running max.  Output is stored bf16
and upcast on host.
"""

import numpy as np

import concourse.mybir as mybir
import concourse.tile as tile
from concourse import bacc
from concourse.bass_utils import run_bass_kernel_spmd

F32 = mybir.dt.float32
F16 = mybir.dt.float16
BF16 = mybir.dt.bfloat16
F8 = mybir.dt.float8e4
DR = mybir.MatmulPerfMode.DoubleRow

B, T, NP, D = 16, 128, 12, 1024
F = 2 * D                      # 2048
NB = 2                         # batch elements per core
NCORES = 8
NC8 = 8                        # chunks per half (g-half fp16 / f-half fp8)
SCALE = 1.0 / float(np.sqrt(F))

_CACHE = {}


def _build():
    nc = bacc.Bacc("TRN2", target_bir_lowering=False, debug=False)
    mult = mybir.AluOpType.mult
    add = mybir.AluOpType.add

    featd = nc.dram_tensor("feat", [NB, T, NP * D], F16, kind="ExternalInput")
    ud = nc.dram_tensor("u", [1, D], F16, kind="ExternalInput")
    mbd = nc.dram_tensor("maskb", [T, T], F32, kind="ExternalInput")
    mad = nc.dram_tensor("maska", [T, T], F32, kind="ExternalInput")
    identd = nc.dram_tensor("ident", [128, 128], F16, kind="ExternalInput")
    # weights: [quarter, part, chunk, 512 e-cols]; 16 = g-rows, 8 = f-rows
    wg_b16 = nc.dram_tensor("wg_b16", [4, 128, NC8, 512], F16, kind="ExternalInput")
    wg_b8 = nc.dram_tensor("wg_b8", [4, 128, NC8, 512], F8, kind="ExternalInput")
    wtp_b16 = nc.dram_tensor("wtp_b16", [4, 128, NC8, 512], F16, kind="ExternalInput")
    wtp_b8 = nc.dram_tensor("wtp_b8", [4, 128, NC8, 512], F8, kind="ExternalInput")
    wtp_a16 = nc.dram_tensor("wtp_a16", [4, 128, NC8, 512], F16, kind="ExternalInput")
    wtp_a8 = nc.dram_tensor("wtp_a8", [4, 128, NC8, 512], F8, kind="ExternalInput")
    wg_a16 = nc.dram_tensor("wg_a16", [4, 128, NC8, 512], F16, kind="ExternalInput")
    wg_a8 = nc.dram_tensor("wg_a8", [4, 128, NC8, 512], F8, kind="ExternalInput")
    outd = nc.dram_tensor("out", [NB, T, F], BF16, kind="ExternalOutput")

    with tile.TileContext(nc) as tc:
        with (
            tc.tile_pool(name="consts", bufs=1) as consts,
            tc.tile_pool(name="w16p", bufs=8) as w16p,
            tc.tile_pool(name="w8p", bufs=8) as w8p,
            tc.tile_pool(name="ntpool", bufs=1) as ntpool,
            tc.tile_pool(name="npool", bufs=1) as npool,
            tc.tile_pool(name="gbpool", bufs=1) as gbp,
            tc.tile_pool(name="tppool", bufs=1) as tpp,
            tc.tile_pool(name="aw", bufs=4) as awp,
            tc.tile_pool(name="s6p", bufs=1) as s6p,
            tc.tile_pool(name="sbp", bufs=2) as sbp,
        ):
            ident = consts.tile([128, 128], F16)
            nc.sync.dma_start(out=ident[:], in_=identd[:])
            u_sb = consts.tile([128, D], F16)
            nc.gpsimd.dma_start(out=u_sb[:], in_=ud[:].to_broadcast((128, D)))
            mb_sb = consts.tile([T, T], F32)
            ma_sb = consts.tile([T, T], F32)
            nc.gpsimd.dma_start(out=mb_sb[:], in_=mbd[:])
            nc.gpsimd.dma_start(out=ma_sb[:], in_=mad[:])

            n16 = [
                npool.tile([T, F], F16, tag=f"n{b}", name=f"n{b}")
                for b in range(NB)
            ]
            # transposed n: g-half fp16 chunks + f-half fp8 chunks
            nT16 = ntpool.tile([128, NC8, NB * T], F16)
            nT8 = ntpool.tile([128, NC8, NB * T], F8)
            gb16 = [
                gbp.tile([T, F], F16, tag=f"gb{b}", name=f"gb{b}")
                for b in range(NB)
            ]
            tp2 = tpp.tile([128, 16, NB * T], F16, tag="tp2", name="tp2")

            def load_q(src16, src8, qc, name):
                """One weight quarter: fp16 g-rows (1MB) then fp8
                f-rows (0.5MB), in consumption order, sync queue."""
                w16 = w16p.tile([128, NC8, 512], F16, tag="w16", name=f"{name}_16")
                nc.sync.dma_start(out=w16[:], in_=src16[qc][:])
                w8 = w8p.tile([128, NC8, 512], F8, tag="w8", name=f"{name}_8")
                nc.sync.dma_start(out=w8[:], in_=src8[qc][:])
                return w16, w8

            # ------------- prologue: n = [g | f], nT -------------
            _fill = [0]

            def emit_fillers(pst, cnt):
                """Dummy transposes: keep the PE clock-gate (HAM) open
                while paced work waits on DMA/DVE dependencies."""
                for _ in range(cnt):
                    _fill[0] += 1
                    pw = pst.tile([128, 128], F16, tag="pt", name=f"w{_fill[0]}")
                    nc.tensor.transpose(pw[:], ident[:], ident[:])

            def emit_prologue(b, feat, psg, pst, nfill=8):
                wv = awp.tile([T, NP], F32, tag=f"wv{b}", name=f"wv{b}")
                gps = [
                    psg.tile([T, 512], F32, tag=f"g{h}", name=f"g{b}{h}")
                    for h in range(2)
                ]
                for c in range(NP):
                    scr = awp.tile([T, D], F16, tag="scr", name=f"scr{b}_{c}")
                    # w_c = sum_d feat_c * u  (DVE, fused mult+row-accum)
                    nc.vector.scalar_tensor_tensor(
                        out=scr[:],
                        in0=feat[:, c, :],
                        scalar=1.0,
                        in1=u_sb[:],
                        op0=mult,
                        op1=mult,
                        accum_out=wv[:, c : c + 1],
                    )
                    # diag(w_c) on ACT: Copy(ident * w_c[per-partition])
                    dw = awp.tile([128, 128], F16, tag="dw", name=f"dw{b}_{c}")
                    nc.scalar.mul(dw[:], ident[:], wv[:, c : c + 1])
                    # g += diag(w_c) @ feat_c  (PE, paced with feat DMA)
                    for h in range(2):
                        nc.tensor.matmul(
                            gps[h][:],
                            dw[:],
                            feat[:, c, 512 * h : 512 * (h + 1)],
                            start=(c == 0),
                            stop=(c == NP - 1),
                        )
                    emit_fillers(pst, nfill)
                # f = max_n feat via wide tree (DVE, after feat lands)
                s6 = s6p.tile([T, 6, D], F16, tag="s6", name=f"s6_{b}")
                fD = n16[b][:, D:]
                nc.vector.tensor_max(s6[:], feat[:, 0:6, :], feat[:, 6:12, :])
                nc.vector.tensor_max(s6[:, 0:3, :], s6[:, 0:3, :], s6[:, 3:6, :])
                nc.vector.tensor_max(fD, s6[:, 0, :], s6[:, 1, :])
                nc.vector.tensor_max(fD, fD, s6[:, 2, :])
                # drain g -> n16 (ACT)
                for h in range(2):
                    nc.scalar.copy(
                        n16[b][:, 512 * h : 512 * (h + 1)], gps[h][:]
                    )
                # g-half transposes (ready first), then f-half -> nT8
                for k in range(8):
                    emit_transp(b, k, pst)
                for k in range(8, 16):
                    emit_transp(b, k, pst)

            def emit_transp(b, k, pst):
                pt = pst.tile([128, 128], F16, tag="pt", name=f"pt{b}_{k}")
                nc.tensor.transpose(
                    pt[:], n16[b][:, 128 * k : 128 * (k + 1)], ident[:]
                )
                dst = (
                    nT16[:, k, T * b : T * (b + 1)]
                    if k < 8
                    else nT8[:, k - 8, T * b : T * (b + 1)]
                )
                if k % 2 == 0:
                    nc.vector.tensor_copy(dst, pt[:])
                else:
                    nc.scalar.copy(dst, pt[:])

            def emit_pass2(w16, w8, qc, b, psg2, fast_drain=False):
                """gb[:, qc-quarter] = n_b @ Wg[qc].T  (t-major).
                f-half fp8 DoubleRow first, then g-half fp16."""
                psg = psg2.tile(
                    [128, 512], F32, tag=f"psg{b}", name=f"psg{b}_{qc}"
                )
                for fc in range(NC8):
                    nc.tensor.matmul(
                        psg[:],
                        nT16[:, fc, T * b : T * (b + 1)],
                        w16[:, fc, :],
                        start=(fc == 0),
                        stop=False,
                    )
                for j in range(4):
                    nc.tensor.matmul(
                        psg[:],
                        nT8[:, 2 * j : 2 * j + 2, T * b : T * (b + 1)],
                        w8[:, 2 * j : 2 * j + 2, :],
                        start=False,
                        stop=(j == 3),
                        perf_mode=DR,
                    )
                lo = 512 * qc
                if fast_drain:
                    nc.scalar.copy(gb16[b][:, lo : lo + 256], psg[:, :256])
                    nc.vector.tensor_copy(
                        gb16[b][:, lo + 256 : lo + 512], psg[:, 256:]
                    )
                else:
                    nc.scalar.copy(gb16[b][:, lo : lo + 512], psg[:])

            def emit_pass1_q(w16, w8, qc, ps1p, sfx):
                """tp2 e-cols for one weight quarter (tb: qc 0,1; pb: 2,3)."""
                for e4 in range(4):
                    p1 = ps1p.tile(
                        [128, NB * T], F32, tag="p1", name=f"p1{sfx}_{qc}_{e4}"
                    )
                    for fc in range(NC8):
                        nc.tensor.matmul(
                            p1[:],
                            w16[:, fc, 128 * e4 : 128 * (e4 + 1)],
                            nT16[:, fc, :],
                            start=(fc == 0),
                            stop=False,
                        )
                    for j in range(4):
                        nc.tensor.matmul(
                            p1[:],
                            w8[:, 2 * j : 2 * j + 2, 128 * e4 : 128 * (e4 + 1)],
                            nT8[:, 2 * j : 2 * j + 2, :],
                            start=False,
                            stop=(j == 3),
                            perf_mode=DR,
                        )
                    if e4 % 2 == 0:
                        nc.scalar.copy(tp2[:, 4 * qc + e4, :], p1[:])
                    else:
                        nc.vector.tensor_copy(tp2[:, 4 * qc + e4, :], p1[:])

            def emit_s(b, mask_sb, ps3p, sfx):
                psb = ps3p.tile([T, T], F32, tag="psb", name=f"psb{sfx}{b}")
                for ec in range(8):
                    nc.tensor.matmul(
                        psb[:],
                        tp2[:, 8 + ec, T * b : T * (b + 1)],
                        tp2[:, ec, T * b : T * (b + 1)],
                        start=(ec == 0),
                        stop=(ec == 7),
                    )
                sbm = sbp.tile([T, T], F16, tag="sbm", name=f"sbm{sfx}{b}")
                nc.vector.scalar_tensor_tensor(
                    out=sbm[:],
                    in0=psb[:],
                    scalar=1.0,
                    in1=mask_sb[:],
                    op0=mult,
                    op1=mult,
                )
                return sbm

            def emit_po(b, h4, sbm, first, ps4p, osb, last=False):
                po = ps4p.tile(
                    [T, 512], F32, tag="po", name=f"po{int(first)}_{b}_{h4}"
                )
                if last:
                    for piece in range(4):
                        pl = 512 * h4 + 128 * piece
                        nc.tensor.matmul(
                            po[:, 128 * piece : 128 * (piece + 1)],
                            sbm[:],
                            gb16[b][:, pl : pl + 128],
                            start=True,
                            stop=True,
                        )
                else:
                    nc.tensor.matmul(
                        po[:],
                        sbm[:],
                        gb16[b][:, 512 * h4 : 512 * (h4 + 1)],
                        start=True,
                        stop=True,
                    )
                lo = 512 * h4
                base = n16[b] if first else osb[b]
                if last:
                    for piece in range(4):
                        pl = lo + 128 * piece
                        nc.vector.scalar_tensor_tensor(
                            out=osb[b][:, pl : pl + 128],
                            in0=po[:, 128 * piece : 128 * (piece + 1)],
                            scalar=1.0,
                            in1=base[:, pl : pl + 128],
                            op0=mult,
                            op1=add,
                        )
                        nc.scalar.dma_start(
                            out=outd[b][:, pl : pl + 128],
                            in_=osb[b][:, pl : pl + 128],
                        )
                else:
                    nc.vector.scalar_tensor_tensor(
                        out=osb[b][:, lo : lo + 512],
                        in0=po[:],
                        scalar=1.0,
                        in1=base[:, lo : lo + 512],
                        op0=mult,
                        op1=add,
                    )

            # ---------------- program ----------------
            with (
                tc.tile_pool(name="featp", bufs=1) as featp,
                tc.tile_pool(name="psg", bufs=1, space="PSUM") as psg,
                tc.tile_pool(name="pst", bufs=4, space="PSUM") as pst,
            ):
                feats = []
                srcs = []
                for b in range(NB):
                    feat = featp.tile(
                        [T, NP, D], F16, tag=f"feat{b}", name=f"feat{b}"
                    )
                    feats.append(feat)
                    srcs.append(featd[b].rearrange("p (c d) -> p c d", c=NP))
                # DMA order: feat b0, first pass2-weight quarters, feat
                # b1, rest -- so pass2-b0 can start while feat b1 lands.
                for q in range(4):
                    nc.sync.dma_start(
                        out=feats[0][:, 3 * q : 3 * (q + 1)],
                        in_=srcs[0][:, 3 * q : 3 * (q + 1)],
                    )
                wq_gb = [load_q(wg_b16, wg_b8, qc, f"wgb{qc}") for qc in range(3)]
                for q in range(4):
                    nc.sync.dma_start(
                        out=feats[1][:, 3 * q : 3 * (q + 1)],
                        in_=srcs[1][:, 3 * q : 3 * (q + 1)],
                    )
                wq_gb += [load_q(wg_b16, wg_b8, qc, f"wgb{qc}") for qc in range(3, 4)]

                # HAM warm-up: dummy transposes as soon as ident lands,
                # keeps the PE clock-gate open until the first g matmul.
                emit_fillers(pst, 60)

                with tc.tile_pool(name="psg2", bufs=1, space="PSUM") as psg2:
                    emit_prologue(0, feats[0], psg, pst)
                    # dovetail: pass2-b0 q0-q2 while feat b1 + wg q3
                    # stream; prologue-b1 PE work fills the q3 wait.
                    for qc in range(3):
                        emit_pass2(*wq_gb[qc], qc, 0, psg2)
                    emit_prologue(1, feats[1], psg, pst, nfill=4)
                    emit_pass2(*wq_gb[3], 3, 0, psg2)
                    for qc in range(4):
                        emit_pass2(*wq_gb[qc], qc, 1, psg2)

            with (
                tc.tile_pool(name="opool", bufs=1) as opool,
                tc.tile_pool(name="ps1", bufs=2, space="PSUM") as ps1p,
                tc.tile_pool(name="ps3", bufs=2, space="PSUM") as ps3p,
                tc.tile_pool(name="ps4", bufs=2, space="PSUM") as ps4p,
                tc.tile_pool(name="psg2b", bufs=1, space="PSUM") as psg2b,
            ):
                osb = [
                    opool.tile([T, F], BF16, tag=f"o{b}", name=f"o{b}")
                    for b in range(NB)
                ]
                # pass1 before
                for qc in range(4):
                    w16, w8 = load_q(wtp_b16, wtp_b8, qc, f"wtb{qc}")
                    emit_pass1_q(w16, w8, qc, ps1p, "b")
                sbm_b = [emit_s(b, mb_sb, ps3p, "b") for b in range(NB)]
                for b in range(NB):
                    for h4 in range(4):
                        emit_po(b, h4, sbm_b[b], True, ps4p, osb)
                # pass1 after
                for qc in range(4):
                    w16, w8 = load_q(wtp_a16, wtp_a8, qc, f"wta{qc}")
                    emit_pass1_q(w16, w8, qc, ps1p, "a")
                sbm_a = [emit_s(b, ma_sb, ps3p, "a") for b in range(NB)]
                # pass2 after, with out-after consuming each quarter
                for qc in range(4):
                    wq = load_q(wg_a16, wg_a8, qc, f"wga{qc}")
                    for b in range(NB):
                        emit_pass2(*wq, qc, b, psg2b, fast_drain=(qc == 3))
                        emit_po(b, qc, sbm_a[b], False, ps4p, osb, last=True)

    nc.compile()
    return nc


def _host_prep(features, Wq, Wtb, Wpb, Wgb, Wta, Wpa, Wga):
    import ml_dtypes

    f32 = np.float32
    f16 = np.float16
    f8 = ml_dtypes.float8_e4m3
    feat = np.ascontiguousarray(np.asarray(features, f32)).reshape(B, T, NP * D)
    u = np.asarray(Wq, f32).sum(axis=0)[None, :]

    def packh(rows, dt):
        # rows: [1024, 2048e] -> [4 qc, 128 p, 8 fc, 512 e]
        a = rows.reshape(NC8, 128, 4, 512).transpose(2, 1, 0, 3)
        return np.ascontiguousarray(a.astype(dt))

    def packs(wt):  # [f, e] fp32 -> (fp16 g-rows pack, fp8 f-rows pack)
        return packh(wt[:D], f16), packh(wt[D:], f8)

    def wt(w):  # [e, f] -> [f, e]
        return np.asarray(w, f32).T

    wtp_b16, wtp_b8 = packs(np.concatenate([wt(Wtb), wt(Wpb)], axis=1))
    wtp_a16, wtp_a8 = packs(np.concatenate([wt(Wta), wt(Wpa)], axis=1))
    wg_b16, wg_b8 = packs(wt(Wgb))
    wg_a16, wg_a8 = packs(wt(Wga))

    idx = np.arange(T)
    maskb = (SCALE * (idx[None, :] > idx[:, None])).astype(f32)  # [j, i]
    maska = (SCALE * (idx[None, :] < idx[:, None])).astype(f32)
    ident = np.eye(128, dtype=f16)

    shared = {
        "u": u.astype(f16),
        "maskb": maskb,
        "maska": maska,
        "ident": ident,
        "wtp_b16": wtp_b16,
        "wtp_b8": wtp_b8,
        "wg_b16": wg_b16,
        "wg_b8": wg_b8,
        "wtp_a16": wtp_a16,
        "wtp_a8": wtp_a8,
        "wg_a16": wg_a16,
        "wg_a8": wg_a8,
    }
    feat16 = feat.astype(f16).reshape(NCORES, NB, T, NP * D)
    return shared, feat16


def kernel(**inputs) -> np.ndarray:
    if "nc" not in _CACHE:
        _CACHE["nc"] = _build()
    nc = _CACHE["nc"]

    shared, feat16 = _host_prep(**inputs)
    in_maps = [dict(shared, feat=feat16[c]) for c in range(NCORES)]
    res = run_bass_kernel_spmd(nc, in_maps, core_ids=list(range(NCORES)))
    out = np.stack([res.results[c]["out"] for c in range(NCORES)], axis=0)
    return out.reshape(B, T, F).astype(np.float32)


# revision 32
# speedup vs baseline: 1.0672x; 1.0672x over previous
"""BiUTE kernel for Trainium2, 8-core data-parallel over batch.

Math (per batch element b, T=128, N=12, D=1024, F=2D=2048):
  u = Wq.sum(0)                                  [D]
  w[t,n]  = sum_d feat[t,n,d] * u[d]             [T,N]
  g[t,d]  = sum_n w[t,n] * feat[t,n,d]           [T,D]
  f[t,d]  = max_n feat[t,n,d]                    [T,D]
  n = [g | f]                                    [T,F]
  tb = n @ Wtb.T ; pb = n @ Wpb.T ; gb = n @ Wgb.T
  sb = (tb @ pb.T) * scale ; out_b = (sb*lower) @ gb
  (same for 'after' branch with upper mask)
  out = n + out_b + out_a                        [T,F]

Sharding: B=16 split 2 per core across 8 cores; weights replicated.

Precision strategy: the g-half of n has sigma~16.5 vs the f-half's
~1.7, so the f-half of every projection contraction runs in fp8-e4m3
with DoubleRow (double-pumped) matmuls -- 2x PE rate and half the DMA
bytes for those weight rows -- while the g-half stays fp16.  Validated
rel-err ~1.2e-2 vs the 2e-2 gate (fp16 baseline 1.0e-3).

Prologue: w via DVE STT accum, diag(w_c) built on ACT (Copy with
per-partition scale), g accumulated on PE chunk-paced with the feat DMA
stream (keeps HAM warm), f via DVE running max.  Output is stored bf16
and upcast on host.
"""

import numpy as np

import concourse.mybir as mybir
import concourse.tile as tile
from concourse import bacc
from concourse.bass_utils import run_bass_kernel_spmd

F32 = mybir.dt.float32
F16 = mybir.dt.float16
BF16 = mybir.dt.bfloat16
F8 = mybir.dt.float8e4
DR = mybir.MatmulPerfMode.DoubleRow

B, T, NP, D = 16, 128, 12, 1024
F = 2 * D                      # 2048
NB = 2                         # batch elements per core
NCORES = 8
NC8 = 8                        # chunks per half (g-half fp16 / f-half fp8)
SCALE = 1.0 / float(np.sqrt(F))

_CACHE = {}


def _build():
    nc = bacc.Bacc("TRN2", target_bir_lowering=False, debug=False)
    mult = mybir.AluOpType.mult
    add = mybir.AluOpType.add

    featd = nc.dram_tensor("feat", [NB, T, NP * D], F16, kind="ExternalInput")
    ud = nc.dram_tensor("u", [1, D], F16, kind="ExternalInput")
    mbd = nc.dram_tensor("maskb", [T, T], F32, kind="ExternalInput")
    mad = nc.dram_tensor("maska", [T, T], F32, kind="ExternalInput")
    identd = nc.dram_tensor("ident", [128, 128], F16, kind="ExternalInput")
    # weights: [quarter, part, chunk, 512 e-cols]; 16 = g-rows, 8 = f-rows
    wg_b16 = nc.dram_tensor("wg_b16", [4, 128, NC8, 512], F16, kind="ExternalInput")
    wg_b8 = nc.dram_tensor("wg_b8", [4, 128, NC8, 512], F8, kind="ExternalInput")
    wtp_b16 = nc.dram_tensor("wtp_b16", [4, 128, NC8, 512], F16, kind="ExternalInput")
    wtp_b8 = nc.dram_tensor("wtp_b8", [4, 128, NC8, 512], F8, kind="ExternalInput")
    wtp_a16 = nc.dram_tensor("wtp_a16", [4, 128, NC8, 512], F16, kind="ExternalInput")
    wtp_a8 = nc.dram_tensor("wtp_a8", [4, 128, NC8, 512], F8, kind="ExternalInput")
    wg_a16 = nc.dram_tensor("wg_a16", [4, 128, NC8, 512], F16, kind="ExternalInput")
    wg_a8 = nc.dram_tensor("wg_a8", [4, 128, NC8, 512], F8, kind="ExternalInput")
    outd = nc.dram_tensor("out", [NB, T, F], BF16, kind="ExternalOutput")

    with tile.TileContext(nc) as tc:
        with (
            tc.tile_pool(name="consts", bufs=1) as consts,
            tc.tile_pool(name="w16p", bufs=8) as w16p,
            tc.tile_pool(name="w8p", bufs=8) as w8p,
            tc.tile_pool(name="ntpool", bufs=1) as ntpool,
            tc.tile_pool(name="npool", bufs=1) as npool,
            tc.tile_pool(name="gbpool", bufs=1) as gbp,
            tc.tile_pool(name="tppool", bufs=1) as tpp,
            tc.tile_pool(name="aw", bufs=4) as awp,
            tc.tile_pool(name="s6p", bufs=1) as s6p,
            tc.tile_pool(name="sbp", bufs=2) as sbp,
        ):
            ident = consts.tile([128, 128], F16)
            nc.sync.dma_start(out=ident[:], in_=identd[:])
            u_sb = consts.tile([128, D], F16)
            nc.gpsimd.dma_start(out=u_sb[:], in_=ud[:].to_broadcast((128, D)))
            mb_sb = consts.tile([T, T], F32)
            ma_sb = consts.tile([T, T], F32)
            nc.gpsimd.dma_start(out=mb_sb[:], in_=mbd[:])
            nc.gpsimd.dma_start(out=ma_sb[:], in_=mad[:])

            n16 = [
                npool.tile([T, F], F16, tag=f"n{b}", name=f"n{b}")
                for b in range(NB)
            ]
            # transposed n: g-half fp16 chunks + f-half fp8 chunks
            nT16 = ntpool.tile([128, NC8, NB * T], F16)
            nT8 = ntpool.tile([128, NC8, NB * T], F8)
            gb16 = [
                gbp.tile([T, F], F16, tag=f"gb{b}", name=f"gb{b}")
                for b in range(NB)
            ]
            tp2 = tpp.tile([128, 16, NB * T], F16, tag="tp2", name="tp2")

            def load_q(src16, src8, qc, name):
                """One weight quarter: fp16 g-rows (1MB) then fp8
                f-rows (0.5MB), in consumption order, sync queue."""
                w16 = w16p.tile([128, NC8, 512], F16, tag="w16", name=f"{name}_16")
                nc.sync.dma_start(out=w16[:], in_=src16[qc][:])
                w8 = w8p.tile([128, NC8, 512], F8, tag="w8", name=f"{name}_8")
                nc.sync.dma_start(out=w8[:], in_=src8[qc][:])
                return w16, w8

            # ------------- prologue: n = [g | f], nT -------------
            _fill = [0]

            def emit_fillers(pst, cnt):
                """Dummy transposes: keep the PE clock-gate (HAM) open
                while paced work waits on DMA/DVE dependencies."""
                for _ in range(cnt):
                    _fill[0] += 1
                    pw = pst.tile([128, 128], F16, tag="pt", name=f"w{_fill[0]}")
                    nc.tensor.transpose(pw[:], ident[:], ident[:])

            def emit_prologue(b, feat, psg, pst, nfill=8):
                wv = awp.tile([T, NP], F32, tag=f"wv{b}", name=f"wv{b}")
                gps = [
                    psg.tile([T, 512], F32, tag=f"g{h}", name=f"g{b}{h}")
                    for h in range(2)
                ]
                for c in range(NP):
                    scr = awp.tile([T, D], F16, tag="scr", name=f"scr{b}_{c}")
                    # w_c = sum_d feat_c * u  (DVE, fused mult+row-accum)
                    nc.vector.scalar_tensor_tensor(
                        out=scr[:],
                        in0=feat[:, c, :],
                        scalar=1.0,
                        in1=u_sb[:],
                        op0=mult,
                        op1=mult,
                        accum_out=wv[:, c : c + 1],
                    )
                    # diag(w_c) on ACT: Copy(ident * w_c[per-partition])
                    dw = awp.tile([128, 128], F16, tag="dw", name=f"dw{b}_{c}")
                    nc.scalar.mul(dw[:], ident[:], wv[:, c : c + 1])
                    # g += diag(w_c) @ feat_c  (PE, paced with feat DMA)
                    for h in range(2):
                        nc.tensor.matmul(
                            gps[h][:],
                            dw[:],
                            feat[:, c, 512 * h : 512 * (h + 1)],
                            start=(c == 0),
                            stop=(c == NP - 1),
                        )
                    emit_fillers(pst, nfill)
                # f = max_n feat via wide tree (DVE, after feat lands)
                s6 = s6p.tile([T, 6, D], F16, tag="s6", name=f"s6_{b}")
                fD = n16[b][:, D:]
                nc.vector.tensor_max(s6[:], feat[:, 0:6, :], feat[:, 6:12, :])
                nc.vector.tensor_max(s6[:, 0:3, :], s6[:, 0:3, :], s6[:, 3:6, :])
                nc.vector.tensor_max(fD, s6[:, 0, :], s6[:, 1, :])
                nc.vector.tensor_max(fD, fD, s6[:, 2, :])
                # drain g -> n16 (ACT)
                for h in range(2):
                    nc.scalar.copy(
                        n16[b][:, 512 * h : 512 * (h + 1)], gps[h][:]
                    )
                # g-half transposes (ready first), then f-half -> nT8
                for k in range(8):
                    emit_transp(b, k, pst)
                for k in range(8, 16):
                    emit_transp(b, k, pst)

            def emit_transp(b, k, pst):
                pt = pst.tile([128, 128], F16, tag="pt", name=f"pt{b}_{k}")
                nc.tensor.transpose(
                    pt[:], n16[b][:, 128 * k : 128 * (k + 1)], ident[:]
                )
                dst = (
                    nT16[:, k, T * b : T * (b + 1)]
                    if k < 8
                    else nT8[:, k - 8, T * b : T * (b + 1)]
                )
                if k % 2 == 0:
                    nc.vector.tensor_copy(dst, pt[:])
                else:
                    nc.scalar.copy(dst, pt[:])

            def emit_pass2(w16, w8, qc, b, psg2, fast_drain=False):
                """gb[:, qc-quarter] = n_b @ Wg[qc].T  (t-major).
                f-half fp8 DoubleRow first, then g-half fp16."""
                psg = psg2.tile(
                    [128, 512], F32, tag=f"psg{b}", name=f"psg{b}_{qc}"
                )
                for fc in range(NC8):
                    nc.tensor.matmul(
                        psg[:],
                        nT16[:, fc, T * b : T * (b + 1)],
                        w16[:, fc, :],
                        start=(fc == 0),
                        stop=False,
                    )
                for j in range(4):
                    nc.tensor.matmul(
                        psg[:],
                        nT8[:, 2 * j : 2 * j + 2, T * b : T * (b + 1)],
                        w8[:, 2 * j : 2 * j + 2, :],
                        start=False,
                        stop=(j == 3),
                        perf_mode=DR,
                    )
                lo = 512 * qc
                if fast_drain:
                    nc.scalar.copy(gb16[b][:, lo : lo + 256], psg[:, :256])
                    nc.vector.tensor_copy(
                        gb16[b][:, lo + 256 : lo + 512], psg[:, 256:]
                    )
                else:
                    nc.scalar.copy(gb16[b][:, lo : lo + 512], psg[:])

            def emit_pass1_q(w16, w8, qc, ps1p, sfx):
                """tp2 e-cols for one weight quarter (tb: qc 0,1; pb: 2,3)."""
                for e4 in range(4):
                    p1 = ps1p.tile(
                        [128, NB * T], F32, tag="p1", name=f"p1{sfx}_{qc}_{e4}"
                    )
                    for fc in range(NC8):
                        nc.tensor.matmul(
                            p1[:],
                            w16[:, fc, 128 * e4 : 128 * (e4 + 1)],
                            nT16[:, fc, :],
                            start=(fc == 0),
                            stop=False,
                        )
                    for j in range(4):
                        nc.tensor.matmul(
                            p1[:],
                            w8[:, 2 * j : 2 * j + 2, 128 * e4 : 128 * (e4 + 1)],
                            nT8[:, 2 * j : 2 * j + 2, :],
                            start=False,
                            stop=(j == 3),
                            perf_mode=DR,
                        )
                    if e4 % 2 == 0:
                        nc.scalar.copy(tp2[:, 4 * qc + e4, :], p1[:])
                    else:
                        nc.vector.tensor_copy(tp2[:, 4 * qc + e4, :], p1[:])

            def emit_s(b, mask_sb, ps3p, sfx):
                psb = ps3p.tile([T, T], F32, tag="psb", name=f"psb{sfx}{b}")
                for ec in range(8):
                    nc.tensor.matmul(
                        psb[:],
                        tp2[:, 8 + ec, T * b : T * (b + 1)],
                        tp2[:, ec, T * b : T * (b + 1)],
                        start=(ec == 0),
                        stop=(ec == 7),
                    )
                sbm = sbp.tile([T, T], F16, tag="sbm", name=f"sbm{sfx}{b}")
                nc.vector.scalar_tensor_tensor(
                    out=sbm[:],
                    in0=psb[:],
                    scalar=1.0,
                    in1=mask_sb[:],
                    op0=mult,
                    op1=mult,
                )
                return sbm

            def emit_po(b, h4, sbm, first, ps4p, osb, last=False):
                po = ps4p.tile(
                    [T, 512], F32, tag="po", name=f"po{int(first)}_{b}_{h4}"
                )
                if last:
                    for piece in range(2):
                        pl = 512 * h4 + 256 * piece
                        nc.tensor.matmul(
                            po[:, 256 * piece : 256 * (piece + 1)],
                            sbm[:],
                            gb16[b][:, pl : pl + 256],
                            start=True,
                            stop=True,
                        )
                else:
                    nc.tensor.matmul(
                        po[:],
                        sbm[:],
                        gb16[b][:, 512 * h4 : 512 * (h4 + 1)],
                        start=True,
                        stop=True,
                    )
                lo = 512 * h4
                base = n16[b] if first else osb[b]
                if last:
                    for piece in range(2):
                        pl = lo + 256 * piece
                        nc.vector.scalar_tensor_tensor(
                            out=osb[b][:, pl : pl + 256],
                            in0=po[:, 256 * piece : 256 * (piece + 1)],
                            scalar=1.0,
                            in1=base[:, pl : pl + 256],
                            op0=mult,
                            op1=add,
                        )
                        nc.scalar.dma_start(
                            out=outd[b][:, pl : pl + 256],
                            in_=osb[b][:, pl : pl + 256],
                        )
                else:
                    nc.vector.scalar_tensor_tensor(
                        out=osb[b][:, lo : lo + 512],
                        in0=po[:],
                        scalar=1.0,
                        in1=base[:, lo : lo + 512],
                        op0=mult,
                        op1=add,
                    )

            # ---------------- program ----------------
            with (
                tc.tile_pool(name="featp", bufs=1) as featp,
                tc.tile_pool(name="psg", bufs=1, space="PSUM") as psg,
                tc.tile_pool(name="pst", bufs=4, space="PSUM") as pst,
            ):
                feats = []
                srcs = []
                for b in range(NB):
                    feat = featp.tile(
                        [T, NP, D], F16, tag=f"feat{b}", name=f"feat{b}"
                    )
                    feats.append(feat)
                    srcs.append(featd[b].rearrange("p (c d) -> p c d", c=NP))
                # DMA order: feat b0, first pass2-weight quarters, feat
                # b1, rest -- so pass2-b0 can start while feat b1 lands.
                for q in range(4):
                    nc.sync.dma_start(
                        out=feats[0][:, 3 * q : 3 * (q + 1)],
                        in_=srcs[0][:, 3 * q : 3 * (q + 1)],
                    )
                wq_gb = [load_q(wg_b16, wg_b8, qc, f"wgb{qc}") for qc in range(3)]
                for q in range(4):
                    nc.sync.dma_start(
                        out=feats[1][:, 3 * q : 3 * (q + 1)],
                        in_=srcs[1][:, 3 * q : 3 * (q + 1)],
                    )
                wq_gb += [load_q(wg_b16, wg_b8, qc, f"wgb{qc}") for qc in range(3, 4)]

                # HAM warm-up: dummy transposes as soon as ident lands,
                # keeps the PE clock-gate open until the first g matmul.
                emit_fillers(pst, 60)

                with tc.tile_pool(name="psg2", bufs=1, space="PSUM") as psg2:
                    emit_prologue(0, feats[0], psg, pst)
                    # dovetail: pass2-b0 q0-q2 while feat b1 + wg q3
                    # stream; prologue-b1 PE work fills the q3 wait.
                    for qc in range(3):
                        emit_pass2(*wq_gb[qc], qc, 0, psg2)
                    emit_prologue(1, feats[1], psg, pst, nfill=1)
                    emit_pass2(*wq_gb[3], 3, 0, psg2)
                    for qc in range(4):
                        emit_pass2(*wq_gb[qc], qc, 1, psg2)

            with (
                tc.tile_pool(name="opool", bufs=1) as opool,
                tc.tile_pool(name="ps1", bufs=2, space="PSUM") as ps1p,
                tc.tile_pool(name="ps3", bufs=2, space="PSUM") as ps3p,
                tc.tile_pool(name="ps4", bufs=2, space="PSUM") as ps4p,
                tc.tile_pool(name="psg2b", bufs=1, space="PSUM") as psg2b,
            ):
                osb = [
                    opool.tile([T, F], BF16, tag=f"o{b}", name=f"o{b}")
                    for b in range(NB)
                ]
                # pass1 before
                for qc in range(4):
                    w16, w8 = load_q(wtp_b16, wtp_b8, qc, f"wtb{qc}")
                    emit_pass1_q(w16, w8, qc, ps1p, "b")
                sbm_b = [emit_s(b, mb_sb, ps3p, "b") for b in range(NB)]
                for b in range(NB):
                    for h4 in range(4):
                        emit_po(b, h4, sbm_b[b], True, ps4p, osb)
                # pass1 after
                for qc in range(4):
                    w16, w8 = load_q(wtp_a16, wtp_a8, qc, f"wta{qc}")
                    emit_pass1_q(w16, w8, qc, ps1p, "a")
                sbm_a = [emit_s(b, ma_sb, ps3p, "a") for b in range(NB)]
                # pass2 after, with out-after consuming each quarter
                for qc in range(4):
                    wq = load_q(wg_a16, wg_a8, qc, f"wga{qc}")
                    for b in range(NB):
                        emit_pass2(*wq, qc, b, psg2b, fast_drain=(qc == 3))
                        emit_po(b, qc, sbm_a[b], False, ps4p, osb, last=True)

    nc.compile()
    return nc


def _host_prep(features, Wq, Wtb, Wpb, Wgb, Wta, Wpa, Wga):
    import ml_dtypes

    f32 = np.float32
    f16 = np.float16
    f8 = ml_dtypes.float8_e4m3
    feat = np.ascontiguousarray(np.asarray(features, f32)).reshape(B, T, NP * D)
    u = np.asarray(Wq, f32).sum(axis=0)[None, :]

    def packh(rows, dt):
        # rows: [1024, 2048e] -> [4 qc, 128 p, 8 fc, 512 e]
        a = rows.reshape(NC8, 128, 4, 512).transpose(2, 1, 0, 3)
        return np.ascontiguousarray(a.astype(dt))

    def packs(wt):  # [f, e] fp32 -> (fp16 g-rows pack, fp8 f-rows pack)
        return packh(wt[:D], f16), packh(wt[D:], f8)

    def wt(w):  # [e, f] -> [f, e]
        return np.asarray(w, f32).T

    wtp_b16, wtp_b8 = packs(np.concatenate([wt(Wtb), wt(Wpb)], axis=1))
    wtp_a16, wtp_a8 = packs(np.concatenate([wt(Wta), wt(Wpa)], axis=1))
    wg_b16, wg_b8 = packs(wt(Wgb))
    wg_a16, wg_a8 = packs(wt(Wga))

    idx = np.arange(T)
    maskb = (SCALE * (idx[None, :] > idx[:, None])).astype(f32)  # [j, i]
    maska = (SCALE * (idx[None, :] < idx[:, None])).astype(f32)
    ident = np.eye(128, dtype=f16)

    shared = {
        "u": u.astype(f16),
        "maskb": maskb,
        "maska": maska,
        "ident": ident,
        "wtp_b16": wtp_b16,
        "wtp_b8": wtp_b8,
        "wg_b16": wg_b16,
        "wg_b8": wg_b8,
        "wtp_a16": wtp_a16,
        "wtp_a8": wtp_a8,
        "wg_a16": wg_a16,
        "wg_a8": wg_a8,
    }
    feat16 = feat.astype(f16).reshape(NCORES, NB, T, NP * D)
    return shared, feat16


def kernel(**inputs) -> np.ndarray:
    if "nc" not in _CACHE:
        _CACHE["nc"] = _build()
    nc = _CACHE["nc"]

    shared, feat16 = _host_prep(**inputs)
    in_maps = [dict(shared, feat=feat16[c]) for c in range(NCORES)]
    res = run_bass_kernel_spmd(nc, in_maps, core_ids=list(range(NCORES)))
    out = np.stack([res.results[c]["out"] for c in range(NCORES)], axis=0)
    return out.reshape(B, T, F).astype(np.float32)
